# revision 19
# baseline (speedup 1.0000x reference)
"""Trainium2 Bass kernel for nn_Attention_CA (sparse_attention).

Reference computation (NUM_HEADS=8):
    x_pool = avgpool4(kv)                  # [b, 96, 4096]
    q = l2norm(Q.reshape(b, 8, 48, 65536)) # over last axis
    k = v = l2norm(x_pool.reshape(b, 8, 12, 4096))
    k, v tiled 16x along length -> 65536
    attn = softmax(q @ k^T)                # [b, 8, 48, 12]
    out  = attn @ v                        # [b, 8, 48, 65536]
    y    = W_proj @ out                    # 1x1 conv over channels

Algebraic structure exploited:
  * q @ tile(k,16)^T == fold16(q) @ k^T where fold16 sums the 16 length-4096
    chunks of each q row; the q l2-norm becomes a per-row logit scale.
  * Everything downstream of the softmax is 16x periodic: the device only
    produces y_small [2, 384, 4096]; the host tiles it (exact).
  * y_small[b] = W @ P[b] @ kn[b] (P = tiny block-diag softmax matrix).
    Each core computes the partial for its two heads,
    y_part = (W[:, my96] @ P_loc) @ kn_loc -> [384, 4096]; the host sums 4
    partials per batch during the unshard => NO device collectives.
  * kn's own l2-norm never touches the big engines: the Gram matrix of the
    raw pooled kv comes from the PE (reusing the knT transposes), and the
    1/norm ends up as a per-column scale on the logits (kinvrow) and a
    per-partition scale folded into the wpt PSUM->SBUF copy.

Sharding over 8 cores: core i owns batch i//4 and heads {2a, 2a+1} (a=i%4).
Q streams in bf16 (12 MB/core).  Stream-phase engine balance (measured
rates: DVE add 2.7us, DVE STT square 6.1us, scalar square 3.9us, gpsimd add
9us for [96,4096] bf16):
  * fold in THREE independent partial accumulators -- A = {0,5,6,7} (DVE),
    G = {1,2,3,4} (gpsimd), B = {8..15} (DVE).  Logits are linear in the
    fold, so each part contributes its own transposes+matmuls to the same
    PSUM accumulation; the slow gpsimd chain is never on a critical path.
  * squares: 14 on scalar (its only stream work -- no ACT table switches),
    2 on DVE via scalar_tensor_tensor.
  * transposes write 4-packed bf16 PSUM group-tiles; one copy per group.
"""

import numpy as np
import ml_dtypes

BF16 = ml_dtypes.bfloat16
NUM_HEADS = 8
B, C, H, W = 2, 384, 256, 256
HW = H * W           # 65536
L = 4096             # kv length == pooled row length
J = HW // L          # 16 fold chunks
CQ = C // NUM_HEADS  # 48 q rows per head
ROWS = 96            # q rows per core (2 heads)
KR = 24              # pooled kv rows per core (2 heads x 12)
NCORES = 8
GROUP = 4            # cores per batch
EPS = 1e-12
NT = L // 128        # 32 column tiles
NMM = L // 512       # 8 matmul column chunks
NB = C // 128        # 3 output row blocks

_CACHE = {}


def _build():
    import concourse.bacc as bacc
    import concourse.mybir as mybir
    from concourse.tile import TileContext

    f32 = mybir.dt.float32
    bf16 = mybir.dt.bfloat16
    Alu = mybir.AluOpType
    Act = mybir.ActivationFunctionType

    nc = bacc.Bacc(num_devices=NCORES)

    q_in = nc.dram_tensor("q", [ROWS, J, L], bf16, kind="ExternalInput")
    kv_in = nc.dram_tensor("kv", [ROWS, L], bf16, kind="ExternalInput")
    wt_in = nc.dram_tensor("wt", [ROWS, C], bf16, kind="ExternalInput")
    ident_in = nc.dram_tensor("ident", [128, 128], bf16, kind="ExternalInput")
    pool_in = nc.dram_tensor("poolmat", [ROWS, KR], bf16, kind="ExternalInput")
    mask_in = nc.dram_tensor("mask", [ROWS, KR], f32, kind="ExternalInput")
    id24_in = nc.dram_tensor("id24", [KR, KR], f32, kind="ExternalInput")
    ones_in = nc.dram_tensor("ones1", [1, ROWS], f32, kind="ExternalInput")
    y_out = nc.dram_tensor("y", [C, L], bf16, kind="ExternalOutput")

    DVE_SQ = (2, 6, 9)         # square via DVE mul+reduce
    GP_CHAIN = (1, 2, 3, 4)    # fold part G on gpsimd
    A_CHAIN = (0, 5, 6, 7)     # fold part A on DVE
    # B = 8..15 on DVE

    with TileContext(nc) as tc:
        with (
            tc.tile_pool(name="big", bufs=2) as big,
            tc.tile_pool(name="persist", bufs=1) as persist,
            tc.tile_pool(name="small", bufs=2) as small,
            tc.tile_pool(name="psum", bufs=2, space="PSUM") as psum,
        ):
            # ---- constants + kv on the sync queue, ahead of the q chunks
            poolmat = persist.tile([ROWS, KR], bf16)
            nc.scalar.dma_start(out=poolmat, in_=pool_in[:, :])
            ident = persist.tile([128, 128], bf16)
            nc.scalar.dma_start(out=ident, in_=ident_in[:, :])
            mask_sb = persist.tile([ROWS, KR], f32)
            nc.scalar.dma_start(out=mask_sb, in_=mask_in[:, :])
            id24_sb = persist.tile([KR, KR], f32)
            nc.scalar.dma_start(out=id24_sb, in_=id24_in[:, :])
            ones_sb = persist.tile([1, ROWS], f32)
            nc.scalar.dma_start(out=ones_sb, in_=ones_in[:, :])
            wt_sb = persist.tile([ROWS, C], bf16)
            nc.scalar.dma_start(out=wt_sb, in_=wt_in[:, :])
            kv_sb = big.tile([ROWS, L], bf16, tag="kv", bufs=1)
            nc.scalar.dma_start(out=kv_sb, in_=kv_in[:, :])

            kn_raw = persist.tile([KR, L], bf16)
            knT = persist.tile([128, NT, KR], bf16)
            gram = psum.tile([KR, KR], f32, tag="wp", bufs=1)
            gmask = small.tile([KR, KR], f32)
            ksq = small.tile([KR, 1], f32)

            def kv_epilogue():
                # pool copies + knT transposes + Gram; emitted after the
                # first two squares so the stream starts immediately
                for n in range(NMM):
                    pp = psum.tile([KR, 2, 512], f32, tag="big512", bufs=2)
                    nc.tensor.matmul(pp[:, 0, :], lhsT=poolmat,
                                     rhs=kv_sb[:, n * 512:(n + 1) * 512],
                                     start=True, stop=True)
                    nc.scalar.copy(kn_raw[:, n * 512:(n + 1) * 512],
                                   pp[:, 0, :])
                for t0 in range(0, NT, 4):
                    ptg = psum.tile([128, 4, KR], bf16, tag="tp")
                    for t in range(t0, t0 + 4):
                        nc.tensor.transpose(ptg[:, t - t0, :],
                                            kn_raw[:, t * 128:(t + 1) * 128],
                                            ident[:KR, :KR])
                    nc.vector.tensor_copy(
                        knT[:, t0:t0 + 4, :].rearrange("p a k -> p (a k)"),
                        ptg.rearrange("p a k -> p (a k)"))
                for t in range(NT):
                    nc.tensor.matmul(gram, lhsT=knT[:, t, :],
                                     rhs=knT[:, t, :],
                                     start=(t == 0), stop=(t == NT - 1))
                nc.vector.tensor_mul(gmask, gram, id24_sb)
                nc.vector.reduce_sum(ksq, gmask, axis=mybir.AxisListType.X)

            # ---- Q stream ----
            sqparts = persist.tile([ROWS, J], f32)
            accA = persist.tile([ROWS, L], bf16)
            accB = persist.tile([ROWS, L], bf16)
            qfT = persist.tile([128, NT, ROWS], bf16)
            pattn = psum.tile([ROWS, KR], f32, tag="attn", bufs=1)
            part_no = [0]

            def part_logits(acc):
                first = part_no[0] == 0
                last = part_no[0] == 1
                part_no[0] += 1
                for t0 in range(0, NT, 4):
                    ptg = psum.tile([128, 4, ROWS], bf16, tag="tp")
                    for t in range(t0, t0 + 4):
                        nc.tensor.transpose(ptg[:, t - t0, :],
                                            acc[:, t * 128:(t + 1) * 128],
                                            ident[:ROWS, :ROWS])
                    nc.vector.tensor_copy(
                        qfT[:, t0:t0 + 4, :].rearrange("p a k -> p (a k)"),
                        ptg.rearrange("p a k -> p (a k)"))
                    for t in range(t0, t0 + 4):
                        nc.tensor.matmul(pattn, lhsT=qfT[:, t, :],
                                         rhs=knT[:, t, :],
                                         start=(first and t == 0),
                                         stop=(last and t == NT - 1))

            chunks = {}
            for j in range(J):
                c = big.tile([ROWS, L], bf16, tag="chunk", bufs=8,
                             name=f"c{j}")
                nc.sync.dma_start(out=c, in_=q_in[:, j, :])
                chunks[j] = c
                # scalar squares first (the DVE ones go after the fold add)
                if j not in DVE_SQ:
                    scr = big.tile([ROWS, L], bf16, tag="sqscr", bufs=1)
                    nc.scalar.activation(scr, c, Act.Square,
                                         accum_out=sqparts[:, j:j + 1])
                # fold: two DVE half-chains
                acc, base = (accA, 0) if j < 8 else (accB, 8)
                if j == base + 1:
                    nc.vector.tensor_add(acc, chunks[base], c)
                elif j != base:
                    nc.vector.tensor_add(acc, acc, c)
                if j in DVE_SQ:
                    scr = big.tile([ROWS, L], bf16, tag="dvescr", bufs=1)
                    nc.vector.tensor_mul(scr, c, c)
                    nc.vector.reduce_sum(sqparts[:, j:j + 1], scr,
                                         axis=mybir.AxisListType.X)
                if j == 1:
                    kv_epilogue()
                # part completions
                if j == 7:
                    part_logits(accA)
                elif j == 15:
                    part_logits(accB)

            # ---- kn norms -> kinvrow (parallel to the B logits) ----
            knrm = small.tile([KR, 1], f32)
            nc.scalar.sqrt(knrm, ksq)
            nc.vector.tensor_scalar_max(knrm, knrm, EPS)
            kinv = small.tile([KR, 1], f32)
            nc.vector.reciprocal(kinv, knrm)
            kinvT = small.tile([1, KR], f32)
            nc.scalar.dma_start(out=kinvT, in_=kinv)
            pkr = psum.tile([ROWS, KR], f32, tag="wp", bufs=1)
            nc.tensor.matmul(pkr, lhsT=ones_sb, rhs=kinvT,
                             start=True, stop=True)
            kinvrow = small.tile([ROWS, KR], f32)
            nc.vector.tensor_copy(kinvrow, pkr)

            # ---- q row norms ----
            sumsq = small.tile([ROWS, 1], f32)
            nc.vector.reduce_sum(sumsq, sqparts, axis=mybir.AxisListType.X)
            qnrm = small.tile([ROWS, 1], f32)
            nc.scalar.sqrt(qnrm, sumsq)
            nc.vector.tensor_scalar_max(qnrm, qnrm, EPS)
            qinv = small.tile([ROWS, 1], f32)
            nc.vector.reciprocal(qinv, qnrm)

            # ---- softmax: logits = qinv * kinvrow * pattn ----
            e_sb = small.tile([ROWS, KR], f32)
            nc.vector.tensor_scalar(e_sb, pattn, qinv, None, Alu.mult)
            nc.vector.tensor_mul(e_sb, e_sb, kinvrow)
            nc.scalar.activation(e_sb, e_sb, Act.Exp)
            p_sb = small.tile([ROWS, KR], f32)
            nc.vector.tensor_mul(p_sb, e_sb, mask_sb)
            esum = small.tile([ROWS, 1], f32)
            nc.vector.reduce_sum(esum, p_sb, axis=mybir.AxisListType.X)
            einv = small.tile([ROWS, 1], f32)
            nc.vector.reciprocal(einv, esum)
            p16 = small.tile([ROWS, KR], bf16)
            nc.vector.tensor_scalar(p16, p_sb, einv, None, Alu.mult)

            # ---- wpt[24, 384] = P^T @ W[:, my96]^T, kinv folded in ----
            pw = psum.tile([KR, C], f32, tag="wp", bufs=1)
            nc.tensor.matmul(pw, lhsT=p16, rhs=wt_sb, start=True, stop=True)
            wpt = small.tile([KR, C], bf16)
            nc.scalar.activation(wpt, pw, Act.Copy, scale=kinv)

            # ---- y_part[384, 4096] = wpt^T @ kn_raw ----
            y_ap = y_out[:, :].rearrange("(ob p) m -> p ob m", p=128)
            for ob in range(NB):
                y_sb = big.tile([128, L], bf16, tag="ysb", bufs=2)
                for n2 in range(NMM // 2):
                    py = psum.tile([128, 2, 512], f32, tag="big512")
                    for h in range(2):
                        n = 2 * n2 + h
                        nc.tensor.matmul(
                            py[:, h, :],
                            lhsT=wpt[:, ob * 128:(ob + 1) * 128],
                            rhs=kn_raw[:, n * 512:(n + 1) * 512],
                            start=True, stop=True)
                    dst = y_sb[:, n2 * 1024:(n2 + 1) * 1024]
                    psrc = py.rearrange("p a k -> p (a k)")
                    if (4 * ob + n2) % 2 == 0:
                        nc.scalar.copy(dst, psrc)
                    else:
                        nc.vector.tensor_copy(dst, psrc)
                    dma_eng = (nc.sync, nc.scalar)[(4 * ob + n2) % 2]
                    dma_eng.dma_start(
                        out=y_ap[:, ob, n2 * 1024:(n2 + 1) * 1024],
                        in_=y_sb[:, n2 * 1024:(n2 + 1) * 1024])

    if not nc.is_finalized():
        nc.finalize()
    return nc


def _get_nc():
    if "nc" not in _CACHE:
        _CACHE["nc"] = _build()
    return _CACHE["nc"]


def kernel(Q, kv, W_proj, _trace=False):
    from concourse.bass_utils import run_bass_kernel_spmd

    Qh = np.ascontiguousarray(Q, dtype=np.float32).reshape(B * C, J, L)
    Qh = Qh.astype(BF16)
    kvh = np.ascontiguousarray(kv, dtype=np.float32).astype(BF16)
    wt_all = np.ascontiguousarray(
        np.asarray(W_proj, dtype=np.float32).T).astype(BF16)

    ident_np = np.eye(128, dtype=BF16)
    poolmat_np = np.zeros((ROWS, KR), dtype=BF16)
    for k in range(KR):
        poolmat_np[4 * k:4 * k + 4, k] = 0.25
    mask_np = np.zeros((ROWS, KR), dtype=np.float32)
    mask_np[:CQ, :12] = 1.0
    mask_np[CQ:, 12:] = 1.0
    id24_np = np.eye(KR, dtype=np.float32)
    ones_np = np.ones((1, ROWS), dtype=np.float32)

    in_maps = []
    for i in range(NCORES):
        b, a = divmod(i, GROUP)
        sl = slice(96 * a, 96 * a + 96)
        in_maps.append({
            "q": Qh[b * C + 96 * a: b * C + 96 * a + 96],
            "kv": kvh[b, sl],
            "wt": wt_all[sl],
            "ident": ident_np,
            "poolmat": poolmat_np,
            "mask": mask_np,
            "id24": id24_np,
            "ones1": ones_np,
        })

    nc = _get_nc()
    res = run_bass_kernel_spmd(nc, in_maps, core_ids=list(range(NCORES)),
                               trace=_trace)
    _CACHE["last_results"] = res

    y_small = np.zeros((B, C, L), np.float32)
    for i in range(NCORES):
        b = i // GROUP
        y_small[b] += res.results[i]["y"].astype(np.float32)

    out = np.broadcast_to(y_small[:, :, None, :], (B, C, J, L))
    return np.ascontiguousarray(out).reshape(B, C, H, W)
